# revision 1
# baseline (speedup 1.0000x reference)
"""CrossModalAttention Trainium2 kernel, v3.

Data-parallel over B*T = 32 frames -> 4 frames per core on 8 cores.
fp16 on-chip; f32 PSUM/stats.  Host-side algebra (see v2): bk drops, bq
folds into the Q projection evac bias, -5*mw/sqrt(hd) folds into Wq, the
sigmoid bias is the DVE rowmin, bv/bo become a constant row added in
numpy.  The DMA-transpose/DMA serialization in the scheduler makes DMA
op COUNT the scarce resource, so v3 uses one packed qkv load, ONE
whole-frame input transpose, TWO half-frame attn transposes and one
store per frame (5 DMA ops total).
"""

import math

import numpy as np

import concourse.bass as bass
import concourse.bacc as bacc
import concourse.mybir as mybir
import concourse.tile as tile
from concourse import bass_utils

F16 = mybir.dt.float16
F32 = mybir.dt.float32
AF = mybir.ActivationFunctionType
ALU = mybir.AluOpType

B, T, S, D = 2, 16, 512, 512
H, HD = 8, 64
NCORES = 8
FRAMES = B * T // NCORES  # 4 frames per core
NT = D // 128


def _emit(tc, nc, aps):
    qkv, wall, bq5, out = aps

    with tc.tile_pool(name="wpool", bufs=1) as wpool, \
         tc.tile_pool(name="tpool", bufs=2) as tpool, \
         tc.tile_pool(name="projpool", bufs=2) as projpool, \
         tc.tile_pool(name="ppool", bufs=1) as ppool, \
         tc.tile_pool(name="apool", bufs=2) as apool, \
         tc.tile_pool(name="statpool", bufs=16) as statpool, \
         tc.tile_pool(name="sps", bufs=4, space="PSUM") as sps, \
         tc.tile_pool(name="attps", bufs=2, space="PSUM") as attps, \
         tc.tile_pool(name="mmps", bufs=2, space="PSUM") as mmps:

        # ---------------- weights (one-time, one packed DMA) -------------
        walls = wpool.tile([128, 4, NT, 512], F16)  # w, i, n
        bq5_sb = wpool.tile([128, NT], F32)
        wq_sb = walls[:, 0, :, :]
        wk_sb = walls[:, 1, :, :]
        wv_sb = walls[:, 2, :, :]
        wo_sb = walls[:, 3, :, :]

        def load_weights(group):
            sl = slice(2 * group, 2 * group + 2)
            nc.gpsimd.dma_start(walls[:, sl, :, :], wall[:, sl, :, :])
            if group == 0:
                nc.gpsimd.dma_start(bq5_sb[:],
                                    bq5.rearrange("(i p) -> p i", p=128))

        # ---------------- per-frame state ----------------
        def alloc_state(f):
            st = {}
            # transposed: [d%128, stile, tensor, dblk, s%128] -- loaded
            # directly from the host-pre-transposed fp16 DRAM layout
            st["qkvT"] = tpool.tile([128, NT, 3, NT, 128], F16, tag="qkvT",
                                    name=f"qkvT_{f}")
            # projections: [dout%128 (head pair-stacked), pair, s]
            st["qT"] = projpool.tile([128, NT, 512], F16, tag="qT",
                                     name=f"qT_{f}")
            st["kT"] = projpool.tile([128, NT, 512], F16, tag="kT",
                                     name=f"kT_{f}")
            # V natural: [k%128, kblk, dout]
            st["vN"] = projpool.tile([128, NT, 512], F16, tag="vN", bufs=3,
                                     name=f"vN_{f}")
            return st

        def emit_load(f, st):
            nc.gpsimd.dma_start(st["qkvT"][:], qkv[f])

        def _rhs(st, t, i):
            # [din%128, (stile), s%128] strided view == [din, s] for chunk i
            return st["qkvT"][:, :, t, i, :]

        def emit_proj_qk_j(st, j):
            ps = mmps.tile([128, 512], F32, tag="mm", name=f"mmq_{j}")
            for i in range(NT):
                nc.tensor.matmul(
                    ps[:], wq_sb[:, i, 128 * j:128 * j + 128],
                    _rhs(st, 0, i), start=(i == 0), stop=(i == NT - 1))
            nc.scalar.activation(st["qT"][:, j, :], ps[:],
                                 AF.Identity, bias=bq5_sb[:, j:j + 1])
            ps = mmps.tile([128, 512], F32, tag="mm", name=f"mmk_{j}")
            for i in range(NT):
                nc.tensor.matmul(
                    ps[:], wk_sb[:, i, 128 * j:128 * j + 128],
                    _rhs(st, 1, i), start=(i == 0), stop=(i == NT - 1))
            nc.vector.tensor_copy(st["kT"][:, j, :], ps[:])

        def emit_proj_qk(st):
            for j in range(NT):
                emit_proj_qk_j(st, j)

        def emit_proj_v(st):
            for m in range(NT):
                ps = mmps.tile([128, 512], F32, tag="mm", name=f"mmv_{m}")
                for i in range(NT):
                    nc.tensor.matmul(
                        ps[:], st["qkvT"][:, m, 2, i, :],
                        wv_sb[:, i, :], start=(i == 0), stop=(i == NT - 1))
                if m % 2 == 0:
                    nc.scalar.activation(st["vN"][:, m, :], ps[:], AF.Copy)
                else:
                    nc.vector.tensor_copy(st["vN"][:, m, :], ps[:])

        # ---------------- attention ----------------
        def emit_scores(f, st, p2, si, a, norm_eng):
            m5 = statpool.tile([128, 2], F32, tag="m5",
                               name=f"m5_{f}_{si}_{a}")
            rs = statpool.tile([128, 2], F32, tag="rs",
                               name=f"rs_{f}_{si}_{a}")
            rsi = statpool.tile([128, 2], F32, tag="rsi",
                                name=f"rsi_{f}_{si}_{a}")
            for j, h in enumerate((2 * a, 2 * a + 1)):
                lo = 64 * (h % 2)
                s_ps = sps.tile([128, 512], F32, tag="s",
                                name=f"s_{f}_{si}_{h}")
                nc.tensor.matmul(
                    s_ps[:],
                    st["qT"][lo:lo + 64, a, 128 * si:128 * si + 128],
                    st["kT"][lo:lo + 64, a, :], start=True, stop=True)
                nc.vector.tensor_reduce(m5[:, j:j + 1], s_ps[:],
                                        mybir.AxisListType.X, ALU.min)
                nc.scalar.activation(p2[:, si % 2, h, :], s_ps[:],
                                     AF.Sigmoid, bias=m5[:, j:j + 1],
                                     scale=-1.0, accum_out=rs[:, j:j + 1])
            for j, h in enumerate((2 * a, 2 * a + 1)):
                nc.vector.reciprocal(rsi[:, j:j + 1], rs[:, j:j + 1])
                norm_eng.tensor_scalar(p2[:, si % 2, h, :],
                                       p2[:, si % 2, h, :],
                                       rsi[:, j:j + 1], None, ALU.mult)

        def emit_attend_half(f, st, attnT_h, aT, half):
            """All 4 head pairs for one q-half (256 cols); two pairs share
            one psum bank."""
            for a in range(4):
                a_ps = attps.tile([128, 256], F32, tag="att",
                                  name=f"aps_{f}_{half}_{a}")
                for h in (2 * a, 2 * a + 1):
                    lo = 64 * (h % 2)
                    for kb in range(NT):
                        nc.tensor.matmul(
                            a_ps[lo:lo + 64, :],
                            st["vN"][:, kb, 64 * h:64 * h + 64],
                            attnT_h[:, :, 4 * h + kb, :],
                            start=(kb == 0), stop=(kb == NT - 1),
                            tile_position=(0, lo))
                nc.vector.tensor_copy(
                    aT[:, a, 256 * half:256 * half + 256], a_ps[:])

        def emit_attend_quarter(f, st, attnT_h, aT, half, q2):
            for a in range(4):
                a_ps = attps.tile([128, 256], F32, tag="att",
                                  name=f"apsq_{f}_{half}_{q2}_{a}")
                for h in (2 * a, 2 * a + 1):
                    lo = 64 * (h % 2)
                    for kb in range(NT):
                        nc.tensor.matmul(
                            a_ps[lo:lo + 64, 0:128],
                            st["vN"][:, kb, 64 * h:64 * h + 64],
                            attnT_h[:, q2, 4 * h + kb, :],
                            start=(kb == 0), stop=(kb == NT - 1),
                            tile_position=(0, lo))
                nc.vector.tensor_copy(
                    aT[:, a, 256 * half + 128 * q2:
                       256 * half + 128 * q2 + 128], a_ps[:, 0:128])

        def emit_outproj(f, aT, outsb, stp):
            ps = mmps.tile([128, 512], F32, tag="mm", name=f"mmo_{f}_{stp}")
            for j in range(NT):
                nc.tensor.matmul(
                    ps[:], aT[:, j, 128 * stp:128 * stp + 128],
                    wo_sb[:, j, :], start=(j == 0), stop=(j == NT - 1))
            if stp % 2 == 0:
                nc.scalar.activation(outsb[:, stp, :], ps[:], AF.Copy)
            else:
                nc.vector.tensor_copy(outsb[:, stp, :], ps[:])
            if stp == NT - 1:
                nc.gpsimd.dma_start(
                    out[f].rearrange("(a p) d -> p a d", p=128), outsb[:])

        # ---------------- schedule ----------------
        st0 = alloc_state(0)
        load_weights(0)
        emit_load(0, st0)
        load_weights(1)
        # prime the sigmoid table set while DMAs run
        warm = wpool.tile([1, 2], F16)
        nc.vector.memset(warm[:], 0.0)
        nc.scalar.activation(warm[:], warm[:], AF.Sigmoid)
        # Q/K projection of frame 0 is interleaved per-j with si0's score
        # pairs inside the main loop (pair a only needs block j=a).

        st = st0
        prev = None  # (f, st, attnT_B, aT, outsb) awaiting half-B tail
        for f in range(FRAMES):
            nxt = alloc_state(f + 1) if f + 1 < FRAMES else None
            p2 = [ppool.tile([128, 2, H, 512], F16, tag=f"p{g}",
                             name=f"p_{f}_{g}") for g in range(2)]
            attnTs = [ppool.tile([128, 2, 32, 128], F16, tag=f"attnT{g}",
                                 name=f"attnT_{f}_{g}") for g in range(2)]
            aT = apool.tile([128, NT, 512], F16, tag="aT", name=f"aT_{f}")
            outsb = apool.tile([128, NT, 512], F16, tag="outsb",
                               name=f"outsb_{f}")
            fillers = {0: [], 1: [], 2: [], 3: []}
            if prev is not None:
                fp, stp, attnTsp, aTp, outsbp = prev
                fillers[0].append(
                    lambda: (emit_attend_half(fp, stp, attnTsp[0], aTp, 0),
                             emit_outproj(fp, aTp, outsbp, 0),
                             emit_outproj(fp, aTp, outsbp, 1)))
                fillers[1].append(
                    lambda: (emit_attend_half(fp, stp, attnTsp[1], aTp, 1),
                             emit_outproj(fp, aTp, outsbp, 2),
                             emit_outproj(fp, aTp, outsbp, 3)))
            if f == 0:
                fillers[0].append(lambda: emit_proj_v(st0))
            if nxt:
                fillers[1].append(lambda: emit_load(f + 1, nxt))
                fillers[2].append(lambda: emit_proj_qk(nxt))
                fillers[3].append(lambda: emit_proj_v(nxt))
            for si in range(NT):
                for a in range(4):
                    if f == 0 and si == 0:
                        emit_proj_qk_j(st, a)
                    on_dve = (si % 2 == 0 and a < 2)
                    norm_eng = nc.vector if on_dve else nc.gpsimd
                    emit_scores(f, st, p2[si // 2], si, a, norm_eng)
                if nxt is None:
                    # last frame: quarter transposes shorten the tail
                    nc.sync.dma_start(attnTs[si // 2][:, si % 2, :, :],
                                      p2[si // 2][:, si % 2, :, :],
                                      transpose=True)
                elif si % 2 == 1:
                    # transpose the completed half (si-1, si)
                    nc.sync.dma_start(attnTs[si // 2][:],
                                      p2[si // 2][:], transpose=True)
                for fn in fillers[si]:
                    fn()
                if nxt is None and si == 2:
                    emit_attend_half(f, st, attnTs[0], aT, 0)
                    emit_outproj(f, aT, outsb, 0)
                    emit_outproj(f, aT, outsb, 1)
                if nxt is None and si == 3:
                    emit_attend_quarter(f, st, attnTs[1], aT, 1, 0)
                    emit_outproj(f, aT, outsb, 2)
            if nxt is None:
                emit_attend_quarter(f, st, attnTs[1], aT, 1, 1)
                emit_outproj(f, aT, outsb, 3)
            prev = (f, st, attnTs, aT, outsb)
            st = nxt


def build_nc():
    nc = bacc.Bacc("TRN2", target_bir_lowering=False, debug=False,
                   num_devices=NCORES)
    qkv = nc.dram_tensor("qkv", (FRAMES, 128, NT, 3, NT, 128), F16,
                         kind="ExternalInput").ap()
    wall = nc.dram_tensor("wall", (128, 4, NT, D), F16,
                          kind="ExternalInput").ap()
    bq5 = nc.dram_tensor("bq5", (D,), F32, kind="ExternalInput").ap()
    out = nc.dram_tensor("out", (FRAMES, S, D), F16,
                         kind="ExternalOutput").ap()
    with tile.TileContext(nc) as tc:
        _emit(tc, nc, (qkv, wall, bq5, out))
    nc.compile()
    return nc


_NC = None


def _get_nc():
    global _NC
    if _NC is None:
        _NC = build_nc()
    return _NC


def make_in_maps(query_spikes, key_spikes, value_spikes, Wq, bq, Wk, bk,
                 Wv, bv, Wo, bo, modality_weights, temporal_sync,
                 query_modality, key_modality):
    qm = int(query_modality)
    km = int(key_modality)
    mw = np.asarray(modality_weights, np.float32)
    c = (mw[qm] * mw[km]) / np.float32(math.sqrt(HD))  # [H]
    scale_cols = np.repeat(-5.0 * c, HD).astype(np.float32)  # [D]
    wq_s = np.asarray(Wq, np.float32) * scale_cols[None, :]
    bq5 = (np.asarray(bq, np.float32) * scale_cols).astype(np.float32)

    f16 = lambda a: np.asarray(a, np.float32).astype(np.float16)
    # wall[p, w, i, n] = W_w[i*128+p, n]
    wall = np.stack([f16(wq_s), f16(Wk), f16(Wv), f16(Wo)])  # [4, D, D]
    wall = np.ascontiguousarray(
        wall.reshape(4, NT, 128, D).transpose(2, 0, 1, 3))
    shared = {"wall": wall, "bq5": bq5}
    # qkv_all[f, dp, st, t, db, sl] = tensor_t[f, st*128+sl, db*128+dp]
    qkv_all = np.stack([
        np.asarray(query_spikes, np.float32).reshape(B * T, S, D),
        np.asarray(key_spikes, np.float32).reshape(B * T, S, D),
        np.asarray(value_spikes, np.float32).reshape(B * T, S, D),
    ], axis=1).astype(np.float16)  # [B*T, 3, S, D]
    qkv_all = qkv_all.reshape(B * T, 3, NT, 128, NT, 128).transpose(
        0, 5, 2, 1, 4, 3)
    in_maps = []
    for core in range(NCORES):
        sl = slice(core * FRAMES, (core + 1) * FRAMES)
        in_maps.append({
            "qkv": np.ascontiguousarray(qkv_all[sl]),
            **shared,
        })
    return in_maps


def host_bias(Wv_np, bv_np, Wo_np, bo_np):
    return (np.asarray(bv_np, np.float64) @ np.asarray(Wo_np, np.float64)
            + np.asarray(bo_np, np.float64)).astype(np.float32)


def kernel(**inputs):
    nc = _get_nc()
    in_maps = make_in_maps(**inputs)
    res = bass_utils.run_bass_kernel_spmd(
        nc, in_maps, core_ids=list(range(NCORES)))
    out = np.concatenate([np.asarray(r["out"], np.float16).astype(np.float32)
                          for r in res.results], axis=0)
    out += host_bias(inputs["Wv"], inputs["bv"], inputs["Wo"], inputs["bo"])
    return out.reshape(B, T, S, D)

